# revision 16
# baseline (speedup 1.0000x reference)
"""Trainium2 Bass kernel for a 3-layer dense-adjacency GCN decoder.

Problem (per batch graph): 3x GCN layer (msg = h@W + b; agg = A @ msg; relu)
followed by output projection + node mask. B=8 graphs of N=2048 nodes,
latent=64, hidden=128, out=64. Batch-parallel: one graph per NeuronCore.

Per-core plan:
  - A (16 MB fp32) is DMA'd once, PE-transposed once into an SBUF-resident
    A^T (the aggregation contracts over A's column index, which must live
    on partitions), and reused by all 3 layers.
  - Features are kept feature-major (h^T: [d, n], fp32). msg^T = W
    (stationary) @ h^T (moving, free-dim 512, float32r single-pass rate);
    bias rides the PSUM->SBUF evacuation as a per-partition ACT bias; PE
    transposes turn msg^T into node-major msg chunks for the aggregation.
  - Aggregation: lhsT = msg chunk (stationary), rhs = A^T slice (moving,
    512 wide) accumulating 16 chunks into PSUM; ReLU rides the evacuation
    and lands directly in the next layer's h^T.
  - Output projection mirrors the msg path; the node mask is a
    per-partition ACT scale on the final transposed copy.
  - variant "bfloat16": A is cast to bf16 during the DMA (SWDGE), A^T and
    msg chunks are bf16 (half SBUF, ~2x faster transposes, FWL weight
    loads); h and the W-matmuls stay f32r.  variant "float32r": A^T/msg
    stored fp32, PE reads them as f32r (TF32-like).  variant "float32":
    everything fp32 (slow, reference-accurate).
"""

import functools
import os

import numpy as np

import concourse.bass as bass
import concourse.bacc as bacc
import concourse.tile as tile
from concourse import mybir
from concourse.bass_utils import run_bass_kernel_spmd

B = 8
N = 2048
NT = N // 128  # 16 partition tiles
LAT = 64
HID = 128
ODIM = 64
N_CORES = 8

F32 = mybir.dt.float32
BF16 = mybir.dt.bfloat16
F32R = mybir.dt.float32r
Act = mybir.ActivationFunctionType


def _c(ap, dt):
    """View an fp32 AP as `dt` for the PE (same 4-byte storage)."""
    return ap if dt == F32 else ap.bitcast(dt)


@functools.lru_cache(maxsize=4)
def _build(variant: str):
    bf16 = variant == "bfloat16"
    # dtype of the A^T / msg storage (the aggregation operands)
    sdt = BF16 if bf16 else F32
    # dtype the W-stationary (msg/proj) matmuls run at
    wdt = F32 if variant == "float32" else F32R
    # dtype the aggregation matmul reads its operands as
    adt = getattr(mybir.dt, variant)

    nc = bacc.Bacc(None, target_bir_lowering=False, debug=False)

    A_d = nc.declare_dram_parameter("A", [N, N], F32, isOutput=False)
    X_d = nc.declare_dram_parameter("X", [N, LAT], F32, isOutput=False)
    MSK_d = nc.declare_dram_parameter("MSK", [NT, 128], F32, isOutput=False)
    W0_d = nc.declare_dram_parameter("W0", [LAT, HID], F32, isOutput=False)
    W1_d = nc.declare_dram_parameter("W1", [HID, HID], F32, isOutput=False)
    W2_d = nc.declare_dram_parameter("W2", [HID, HID], F32, isOutput=False)
    WO_d = nc.declare_dram_parameter("WO", [HID, ODIM], F32, isOutput=False)
    B0_d = nc.declare_dram_parameter("B0", [HID, 1], F32, isOutput=False)
    B1_d = nc.declare_dram_parameter("B1", [HID, 1], F32, isOutput=False)
    B2_d = nc.declare_dram_parameter("B2", [HID, 1], F32, isOutput=False)
    BO_d = nc.declare_dram_parameter("BO", [ODIM, 1], F32, isOutput=False)
    ID_d = nc.declare_dram_parameter("ID", [128, 128], F32, isOutput=False)
    Y_d = nc.declare_dram_parameter("Y", [N, ODIM], F32, isOutput=True)
    Y3 = Y_d[:].rearrange("(t p) f -> p t f", p=128)

    with tile.TileContext(nc) as tc:
        with (
            tc.tile_pool(name="const", bufs=1) as constp,
            tc.tile_pool(name="at", bufs=1) as atp,
            tc.tile_pool(name="stage", bufs=8 if bf16 else 4) as stagep,
            tc.tile_pool(name="ht", bufs=2) as htp,
            tc.tile_pool(name="msg", bufs=2) as msgp,
            tc.tile_pool(name="msgt", bufs=2) as msgtp,
            tc.tile_pool(name="xo", bufs=1) as xop,
            tc.tile_pool(name="tp", bufs=2, space=bass.MemorySpace.PSUM) as tpp,
            tc.tile_pool(name="aggp", bufs=4, space=bass.MemorySpace.PSUM) as aggp,
            tc.tile_pool(name="msum", bufs=2, space=bass.MemorySpace.PSUM) as msump,
        ):
            at_t = atp.tile([128, NT * N], sdt, tag="at")
            at3 = at_t[:].rearrange("p (j c) -> p j c", c=N)

            # ---- SWDGE queue: ident first (gates all transposes), the
            # first A tiles next, then constants interleaved by need ----
            ident = constp.tile([128, 128], F32, tag="ident")
            nc.gpsimd.dma_start(ident[:], ID_d[:])

            stage = {}

            def _stage_bf16(t):
                st = stagep.tile([128, N], BF16, tag="stage", name=f"st{t}")
                nc.gpsimd.dma_start(st[:], A_d[t * 128 : (t + 1) * 128, :])
                stage[t] = st

            if bf16:
                _stage_bf16(0)
                _stage_bf16(1)

            xn_t = xop.tile([128, NT * LAT], F32, tag="xo")
            nc.gpsimd.dma_start(
                xn_t[:].rearrange("p (t f) -> p t f", f=LAT),
                X_d[:].rearrange("(t p) f -> p t f", p=128),
            )
            w0_t = constp.tile([LAT, HID], F32, tag="w0")
            nc.gpsimd.dma_start(w0_t[:], W0_d[:])
            b0_t = constp.tile([HID, 1], F32, tag="b0")
            nc.gpsimd.dma_start(b0_t[:], B0_d[:])
            ident_s = ident
            if bf16:
                ident_s = constp.tile([128, 128], BF16, tag="idents")
                nc.vector.tensor_copy(ident_s[:], ident[:])
                for t in range(2, NT):
                    _stage_bf16(t)
            else:
                for i4 in range(4):
                    for h in range(2):
                        for tt in range(4 * i4, 4 * i4 + 4):
                            st = stagep.tile([128, N // 2], F32, tag="stage",
                                             name=f"st{tt}_{h}")
                            nc.sync.dma_start(
                                st[:], A_d[tt * 128 : (tt + 1) * 128,
                                           h * 1024 : (h + 1) * 1024]
                            )
                            stage[(tt, h)] = st
            w1_t = constp.tile([HID, HID], F32, tag="w1")
            nc.gpsimd.dma_start(w1_t[:], W1_d[:])
            b1_t = constp.tile([HID, 1], F32, tag="b1")
            nc.gpsimd.dma_start(b1_t[:], B1_d[:])
            w2_t = constp.tile([HID, HID], F32, tag="w2")
            nc.gpsimd.dma_start(w2_t[:], W2_d[:])
            b2_t = constp.tile([HID, 1], F32, tag="b2")
            nc.gpsimd.dma_start(b2_t[:], B2_d[:])
            wo_t = constp.tile([HID, ODIM], F32, tag="wo")
            nc.gpsimd.dma_start(wo_t[:], WO_d[:])
            bo_t = constp.tile([ODIM, 1], F32, tag="bo")
            nc.gpsimd.dma_start(bo_t[:], BO_d[:])
            mskn_t = constp.tile([NT, 128], F32, tag="mskn")
            nc.gpsimd.dma_start(mskn_t[:], MSK_d[:])

            # round the stationary weights once for the f32r matmuls
            w0_r = constp.tile([LAT, HID], F32, tag="w0r")
            nc.vector.tensor_copy(_c(w0_r[:], wdt), w0_t[:])
            w1_r = constp.tile([HID, HID], F32, tag="w1r")
            nc.vector.tensor_copy(_c(w1_r[:], wdt), w1_t[:])
            w2_r = constp.tile([HID, HID], F32, tag="w2r")
            nc.vector.tensor_copy(_c(w2_r[:], wdt), w2_t[:])
            wo_r = constp.tile([HID, ODIM], F32, tag="wor")
            nc.vector.tensor_copy(_c(wo_r[:], wdt), wo_t[:])

            # mask^T: [128, NT], column t holds mask[t*128 : (t+1)*128]
            mskT = constp.tile([128, NT], F32, tag="mskT")

            def emit_maskT():
                with nc.named_scope("maskT"):
                    mps = tpp.tile([128, 512], F32, tag="tp", name="mask_ps")
                    nc.tensor.transpose(
                        mps[0:128, 0:NT], mskn_t[:], ident[0:NT, 0:NT]
                    )
                    nc.vector.tensor_copy(mskT[:], mps[0:128, 0:NT])

            # X^T -> h0^T  [LAT, N]
            hT0 = htp.tile([128, N], F32, tag="ht")

            def emit_xT():
                with nc.named_scope("xT"):
                    for g in range(4):
                        ps = tpp.tile([128, 512], F32, tag="tp", name=f"xtp{g}")
                        for q in range(4):
                            t = 4 * g + q
                            nc.tensor.transpose(
                                ps[0:LAT, q * 128 : (q + 1) * 128],
                                xn_t[:, t * LAT : (t + 1) * LAT],
                                ident[:],
                            )
                        nc.vector.tensor_copy(
                            _c(hT0[0:LAT, g * 512 : (g + 1) * 512], wdt),
                            ps[0:LAT, 0:512],
                        )

            msgT_tiles = {}

            def emit_msgT(lname, i, hT, din, w_r, b_col):
                """Stage 1: msg^T chunk i = (W stationary) @ h^T[:, i-slice]
                into PSUM, bias on the ACT evacuation to SBUF."""
                with nc.named_scope(lname):
                    mp = msump.tile([128, 512], F32, tag="msum",
                                    name=f"{lname}_mp{i}")
                    nc.tensor.matmul(
                        mp[0:HID, :],
                        _c(w_r[0:din, :], wdt),
                        _c(hT[0:din, i * 512 : (i + 1) * 512], wdt),
                        start=True,
                        stop=True,
                    )
                    msgT = msgtp.tile([128, 512], sdt, tag="msgt",
                                      name=f"{lname}_msgT{i}")
                    nc.scalar.activation(
                        msgT[:], mp[0:HID, :], Act.Identity, bias=b_col[:]
                    )
                    msgT_tiles[(lname, i)] = msgT

            def emit_msg_tp(lname, i, msg_nat):
                """Stage 2: PE-transpose msg^T chunk i into node-major."""
                with nc.named_scope(lname):
                    msgT = msgT_tiles.pop((lname, i))
                    ps = tpp.tile([128, 512], sdt, tag="tp",
                                  name=f"{lname}_tp{i}")
                    for q in range(4):
                        nc.tensor.transpose(
                            ps[:, q * 128 : (q + 1) * 128],
                            msgT[:, q * 128 : (q + 1) * 128],
                            ident_s[:],
                        )
                    nc.vector.tensor_copy(
                        _c(msg_nat[:, i * 512 : (i + 1) * 512], adt), ps[:]
                    )

            def emit_msg_chunk(lname, i, hT, din, w_r, b_col, msg_nat):
                emit_msgT(lname, i, hT, din, w_r, b_col)
                emit_msg_tp(lname, i, msg_nat)

            projT_tiles = {}

            def emit_projT(i, hT):
                with nc.named_scope("proj"):
                    pp = msump.tile([128, 512], F32, tag="msum", name=f"pp{i}")
                    nc.tensor.matmul(
                        pp[0:ODIM, :],
                        _c(wo_r[:], wdt),
                        _c(hT[:, i * 512 : (i + 1) * 512], wdt),
                        start=True,
                        stop=True,
                    )
                    projT = msgtp.tile([128, 512], F32, tag="msgt",
                                       name=f"projT{i}")
                    nc.scalar.activation(
                        projT[0:ODIM, :], pp[0:ODIM, :], Act.Identity,
                        bias=bo_t[:],
                    )
                    projT_tiles[i] = projT

            def emit_proj_out(i, out_sb):
                """Transpose proj^T chunk i back node-major with the mask
                as ACT scale, then stream the Y DMA."""
                with nc.named_scope("proj"):
                    projT = projT_tiles.pop(i)
                    ps = tpp.tile([128, 512], F32, tag="tp", name=f"otp{i}")
                    for q in range(4):
                        nc.tensor.transpose(
                            ps[:, q * 64 : (q + 1) * 64],
                            projT[0:ODIM, q * 128 : (q + 1) * 128],
                            ident[0:ODIM, 0:ODIM],
                        )
                    for q in range(4):
                        j = 4 * i + q
                        nc.scalar.activation(
                            out_sb[:, j * ODIM : (j + 1) * ODIM],
                            ps[:, q * 64 : (q + 1) * 64],
                            Act.Copy,
                            scale=mskT[:, j : j + 1],
                        )
                    nc.sync.dma_start(
                        Y3[:, 4 * i : 4 * i + 4, :],
                        out_sb[:].rearrange("p (t f) -> p t f", f=ODIM)[
                            :, 4 * i : 4 * i + 4, :
                        ],
                    )

            def _agg_mm(ap_ps, i, j, msg_nat):
                nc.tensor.matmul(
                    ap_ps[:],
                    _c(msg_nat[:, j * 128 : (j + 1) * 128], adt),
                    _c(at_t[:, j * N + i * 512 : j * N + (i + 1) * 512], adt),
                    start=(j == 0),
                    stop=(j == NT - 1),
                )

            def emit_prep_half_f32(tt, h):
                """fp32 path: transpose staged half (tt, h) into A^T."""
                with nc.named_scope(f"prep{tt}_{h}"):
                    st = stage[(tt, h)]
                    for j2 in (2 * h, 2 * h + 1):
                        ps = tpp.tile([128, 512], F32, tag="tp",
                                      name=f"prep{tt}_{h}_{j2}")
                        for q in range(4):
                            nc.tensor.transpose(
                                ps[:, q * 128 : (q + 1) * 128],
                                st[:, (j2 - 2 * h) * 512 + q * 128 :
                                   (j2 - 2 * h) * 512 + (q + 1) * 128],
                                ident[:],
                            )
                        src_ap = ps[:].rearrange("p (q c) -> p q c", c=128)
                        dst = _c(
                            at3[:, 4 * j2 : 4 * j2 + 4,
                                tt * 128 : (tt + 1) * 128],
                            adt,
                        )
                        if j2 % 2 == 0:
                            nc.vector.tensor_copy(dst, src_ap)
                        else:
                            nc.scalar.copy(dst, src_ap)

            def emit_prep_row_bf16(t):
                """bf16 path: transpose staged row-tile t into A^T."""
                with nc.named_scope(f"prep{t}"):
                    st = stage[t]
                    for half in range(2):
                        ps = tpp.tile([128, 1024], BF16, tag="tp",
                                      name=f"prep{t}_{half}")
                        for q in range(8):
                            j = 8 * half + q
                            nc.tensor.transpose(
                                ps[:, q * 128 : (q + 1) * 128],
                                st[:, j * 128 : (j + 1) * 128],
                                ident_s[:],
                            )
                        src_ap = ps[:].rearrange("p (q c) -> p q c", c=128)
                        dst = at3[:, 8 * half : 8 * half + 8,
                                  t * 128 : (t + 1) * 128]
                        if half == 0 or t % 2 == 0:
                            nc.vector.tensor_copy(dst, src_ap)
                        else:
                            nc.scalar.copy(dst, src_ap)

            # ---- fully pipelined layer chain ----
            # Layer 0 weaves A^T prep (DMA-paced) between agg work; every
            # agg chunk ReLUs into the next h^T slice; the next layer's msg
            # chunk (or proj chunk) is emitted one chunk late so the
            # ACT-side ReLU never stalls the PE.
            msg_cur = msgp.tile([128, N], sdt, tag="msg", name="msg0")
            out_sb = xop.tile([128, NT * ODIM], F32, tag="xo", name="out_sb")
            hT_cur = hT0
            w_next = ((w1_r, b1_t), (w2_r, b2_t))
            for l in range(3):
                hT_next = htp.tile([128, N], F32, tag="ht", name=f"hT{l + 1}")
                if l < 2:
                    msg_next = msgp.tile([128, N], sdt, tag="msg",
                                         name=f"msg{l + 1}")
                for i in range(4):
                    ap_ps = aggp.tile([128, 512], F32, tag="agg",
                                      name=f"agg{l}_{i}")
                    if l == 0 and bf16:
                        for t in range(4 * i, 4 * i + 4):
                            emit_prep_row_bf16(t)
                            if i == 0 and t == 1:
                                emit_xT()
                                for g in range(4):
                                    emit_msg_chunk("msg0", g, hT0, LAT,
                                                   w0_r, b0_t, msg_cur)
                        with nc.named_scope("agg0"):
                            for j in range(NT):
                                _agg_mm(ap_ps, i, j, msg_cur)
                    elif l == 0:
                        for h in range(2):
                            for tt in range(4 * i, 4 * i + 4):
                                emit_prep_half_f32(tt, h)
                            if i == 0 and h == 0:
                                emit_xT()
                                for g in range(4):
                                    emit_msg_chunk("msg0", g, hT0, LAT,
                                                   w0_r, b0_t, msg_cur)
                            with nc.named_scope("agg0"):
                                for j in range(8 * h, 8 * h + 8):
                                    _agg_mm(ap_ps, i, j, msg_cur)
                    else:
                        with nc.named_scope(f"agg{l}"):
                            for j in range(NT):
                                _agg_mm(ap_ps, i, j, msg_cur)
                    with nc.named_scope(f"relu{l}"):
                        nc.scalar.activation(
                            _c(hT_next[:, i * 512 : (i + 1) * 512], wdt),
                            ap_ps[:],
                            Act.Relu,
                        )
                    # two-stage-delayed follow-on work: the W-matmul for
                    # chunk i-1 runs after agg chunk i (its ReLU is done by
                    # then), and its transposes one agg chunk later still
                    # (the ACT-side evacuation is done by then).
                    if i >= 1:
                        if l < 2:
                            emit_msgT(f"msg{l + 1}", i - 1, hT_next, HID,
                                      w_next[l][0], w_next[l][1])
                        else:
                            if i == 1:
                                emit_maskT()
                            emit_projT(i - 1, hT_next)
                    if i >= 2:
                        if l < 2:
                            emit_msg_tp(f"msg{l + 1}", i - 2, msg_next)
                        else:
                            emit_proj_out(i - 2, out_sb)
                if l < 2:
                    emit_msgT(f"msg{l + 1}", 3, hT_next, HID,
                              w_next[l][0], w_next[l][1])
                    emit_msg_tp(f"msg{l + 1}", 2, msg_next)
                    emit_msg_tp(f"msg{l + 1}", 3, msg_next)
                else:
                    emit_projT(3, hT_next)
                    emit_proj_out(2, out_sb)
                    emit_proj_out(3, out_sb)
                hT_cur = hT_next
                if l < 2:
                    msg_cur = msg_next

    nc.compile()
    return nc


_EYE = np.eye(128, dtype=np.float32)


def kernel(
    latent_features,
    adjacency_matrix,
    node_mask,
    W0,
    b0,
    W1,
    b1,
    W2,
    b2,
    Wout,
    bout,
    _trace=False,
    _agg_dt=None,
):
    variant = _agg_dt or os.environ.get("GCN_AGG_DT", "float32r")
    nc = _build(variant)

    lat = np.ascontiguousarray(np.asarray(latent_features, dtype=np.float32))
    adj = np.ascontiguousarray(np.asarray(adjacency_matrix, dtype=np.float32))
    msk = np.ascontiguousarray(np.asarray(node_mask, dtype=np.float32))
    w0 = np.ascontiguousarray(np.asarray(W0, dtype=np.float32))
    w1 = np.ascontiguousarray(np.asarray(W1, dtype=np.float32))
    w2 = np.ascontiguousarray(np.asarray(W2, dtype=np.float32))
    wo = np.ascontiguousarray(np.asarray(Wout, dtype=np.float32))
    b0_ = np.asarray(b0, dtype=np.float32).reshape(HID, 1)
    b1_ = np.asarray(b1, dtype=np.float32).reshape(HID, 1)
    b2_ = np.asarray(b2, dtype=np.float32).reshape(HID, 1)
    bo_ = np.asarray(bout, dtype=np.float32).reshape(ODIM, 1)

    in_maps = []
    for c in range(N_CORES):
        in_maps.append(
            {
                "A": adj[c],
                "X": lat[c],
                "MSK": msk[c].reshape(NT, 128),
                "W0": w0,
                "W1": w1,
                "W2": w2,
                "WO": wo,
                "B0": b0_,
                "B1": b1_,
                "B2": b2_,
                "BO": bo_,
                "ID": _EYE,
            }
        )

    res = run_bass_kernel_spmd(
        nc, in_maps, core_ids=list(range(N_CORES)), trace=_trace
    )
    out = np.stack([res.results[c]["Y"] for c in range(N_CORES)], axis=0)
    if _trace:
        return out, res
    return out
